# revision 19
# baseline (speedup 1.0000x reference)
"""MemoryReader kernel for Trainium2, data-parallel over batch across 8 cores.

Per batch element b (one NeuronCore each):
    mkf = mk[b] as [CK=64, M=4096], qkf = qk[b] as [CK, N=4096]
    aff[m, n] = (2 * mkf.T @ qkf - |mkf[:,m]|^2) / sqrt(CK)
    P = softmax over m
    mem[c, n]  = sum_m mv[b][c, m] * P[m, n]
    out[b] = concat([mem, qv[b]], channel axis)

Device kernel layout (per core), v3 (transposed fp8 DoubleRow readout):
    - QK^T matmuls produce logit chunks in [m-partition, n-free] layout,
      one [128, 512] PSUM bank per m-chunk, 3 rotating banks. The exp
      bias (2 - |mk col|^2/8, the +2 centers fp8 range and cancels in
      the softmax) rides in contraction row 64: mk row 64 holds
      8 - |col|^2/2 and qk row 64 holds 1.0, so the bias comes out of
      the matmul for free and ScalarE's exp needs no bias operand.
    - ScalarE computes E = exp(0.25 * psum) per chunk straight out of
      PSUM, writing float8e4 halves of a [128, 2, 512] pair tile.
    - Readout is TRANSPOSED: for each n-chunk k, out[n, c] accumulates
      lhsT = E-pair[:, :, 128k:128k+128] (stationary) against
      rhs = mv-pair [128, 2, 512] (moving) in fp8 DoubleRow mode
      (256 contraction rows per instruction). Output [n-part, c-free]
      makes the softmax 1/s a per-partition scalar.
    - Denominator: a DoubleRow ones-matmul accumulates s[n] per super in
      one [1, 512] PSUM row; rows 0/32 of one bank alternate between
      supers so the next super never waits on the tail chain.
    - PSUM budget: 3 (qk) + 4 (readout) + 1 (denominator) = 8 banks.
    - Tail per super (emitted a few pairs into the next super): ScalarE
      copies s to SBUF, four K=1 matmuls transpose it to [128, 4], DVE
      reciprocal + per-partition scale of the evacuated readout, DMA out.
    - mv^T is quantized to fp8e4 and laid out host-side. qv never
      touches the device. Output is [N, CV]; host transposes back.
"""

import os
import sys

import numpy as np

B, CK, CV, H, W = 8, 64, 512, 64, 64
M = H * W          # memory positions per batch element
N = H * W          # query positions
NT = 512           # n-super-tile width (columns per softmax pass)
NSUP = N // NT     # 8 n-super-tiles
MCH = M // 128     # 32 m-chunks
PAIRS = MCH // 2   # 16 m-chunk pairs (DoubleRow contracts 256 rows)
N_CORES = 8

_CACHE = {}


def _build_program():
    sys.path.insert(0, "/opt/trn_rl_repo")
    from contextlib import ExitStack

    import concourse.tile as tile
    from concourse import bacc, mybir

    dt = mybir.dt
    f32 = dt.float32
    f32r = dt.float32r
    f8 = dt.float8e4
    DR = mybir.MatmulPerfMode.DoubleRow

    nc = bacc.Bacc("TRN2", target_bir_lowering=False, debug=False,
                   num_devices=N_CORES)

    mk_d = nc.dram_tensor("mk", [128, M], f32r, kind="ExternalInput").ap()
    qk_d = nc.dram_tensor("qk", [128, N], f32r, kind="ExternalInput").ap()
    mvt_d = nc.dram_tensor("mvt", [128, MCH * CV], f8,
                           kind="ExternalInput").ap()
    mem_d = nc.dram_tensor("mem", [N, CV], f32, kind="ExternalOutput").ap()

    with tile.TileContext(nc) as tc, ExitStack() as ctx:
        sing = ctx.enter_context(tc.tile_pool(name="sing", bufs=1))
        e_pool = ctx.enter_context(tc.tile_pool(name="E", bufs=12))
        sacc_pool = ctx.enter_context(tc.tile_pool(name="sacc", bufs=2))
        row_pool = ctx.enter_context(tc.tile_pool(name="row", bufs=2))
        inv_pool = ctx.enter_context(tc.tile_pool(name="inv", bufs=2))
        out_pool = ctx.enter_context(tc.tile_pool(name="out", bufs=8))
        qk_ps_pool = ctx.enter_context(
            tc.tile_pool(name="qkps", bufs=3, space="PSUM"))
        ro_ps_pool = ctx.enter_context(
            tc.tile_pool(name="rops", bufs=1, space="PSUM"))
        s_ps_pool = ctx.enter_context(
            tc.tile_pool(name="sps", bufs=1, space="PSUM"))

        # PE warmup: the PE activity monitor starts throttled at 1.2 GHz
        # and needs ~3.4us of sustained matmul activity to unthrottle.
        # Burn dummy matmuls while the input DMAs stream so the real
        # matmuls start at 2.4 GHz.
        warm_sb = sing.tile([128, 128], f32)
        nc.vector.memset(warm_sb[:], 1.0)
        warm_ps = qk_ps_pool.tile([128, NT], f32, tag="qk_ps", name="warm_ps")
        for w in range(16):
            nc.tensor.matmul(warm_ps[:, 0:128], lhsT=warm_sb[:],
                             rhs=warm_sb[:], start=True, stop=True)

        # Resident inputs. mk/qk carry CK=64 data rows, the exp-bias /
        # ones row at 64, and zero padding to K=128 (K=64 matmuls run at
        # the throttled PE clock, so padded K=128 is 2x faster). DMAs
        # ordered so the tensors gating the first matmuls arrive first.
        mk_sb = sing.tile([128, M], f32r)
        qk_sb = sing.tile([128, N], f32r)
        mvt_sb = sing.tile([128, MCH, CV], f8)
        nc.sync.dma_start(out=qk_sb[:, 0:NT], in_=qk_d[:, 0:NT])
        nc.sync.dma_start(out=mk_sb[:, 0:1024], in_=mk_d[:, 0:1024])
        nc.sync.dma_start(
            out=mvt_sb[:, 0:8, :],
            in_=mvt_d[:, 0:8 * CV].rearrange("p (j c) -> p j c", c=CV))
        for g in range(1, 4):
            gs = slice(g * 1024, (g + 1) * 1024)
            nc.sync.dma_start(out=mk_sb[:, gs], in_=mk_d[:, gs])
        for g in range(1, 4):
            gs = slice(g * 8 * CV, (g + 1) * 8 * CV)
            nc.sync.dma_start(
                out=mvt_sb[:, g * 8:(g + 1) * 8, :],
                in_=mvt_d[:, gs].rearrange("p (j c) -> p j c", c=CV))
        nc.sync.dma_start(out=qk_sb[:, NT:N], in_=qk_d[:, NT:N])

        # Ones operands: fp8 pair-column for the DoubleRow denominator
        # matmul, fp32r single element for the s-transpose matmuls.
        ones_f32 = sing.tile([128, 2, 16], f32)
        nc.vector.memset(ones_f32[:], 1.0)
        ones2 = sing.tile([128, 2, 16], f8)
        with nc.allow_low_precision(reason="exact value 1.0 in fp8"):
            nc.vector.tensor_copy(ones2[:], ones_f32[:])
        one1 = sing.tile([1, 1], f32)
        nc.vector.memset(one1[:], 1.0)
        ones_col_f32 = sing.tile([128, 1], f32)
        nc.vector.memset(ones_col_f32[:], 1.0)
        ones_col = sing.tile([128, 1], f32r)
        nc.vector.tensor_copy(ones_col[:], ones_col_f32[:].bitcast(f32r))

        s_ps = s_ps_pool.tile([1, NT], f32, tag="s_ps", name="s_ps")

        # Softmax-sum split: pairs 0..SPLIT-1 accumulate on VectorE from
        # the fp8 E tiles (sacc), pairs SPLIT..15 on PE via DoubleRow
        # ones-matmuls; a K=128 ones-matmul folds sacc's partition axis
        # into row 32 of the denominator bank.
        SPLIT = 12

        def emit_tail(ti, ts_sb, tosbs, tnsl):
            # 1/s chain and final scaling for super `ti`, emitted a few
            # pairs into the NEXT super so the boundary engines stay
            # clear while the chain resolves.
            st = qk_ps_pool.tile([128, 4], f32, tag="qk_ps",
                                 name=f"st{ti}")
            for k in range(4):
                nc.tensor.matmul(
                    st[:, k:k + 1],
                    lhsT=ts_sb[0:1, k * 128:(k + 1) * 128],
                    rhs=one1[:], start=True, stop=True)
            inv_s = inv_pool.tile([128, 4], f32, tag="inv",
                                  name=f"inv{ti}")
            nc.vector.reciprocal(inv_s[:], st[:])
            with nc.allow_low_precision(reason="fp32 scale of fp32 data"):
                for k in range(4):
                    nc.vector.tensor_scalar_mul(
                        tosbs[k][:], tosbs[k][:], inv_s[:, k:k + 1])
            for k in range(4):
                nc.sync.dma_start(
                    out=mem_d[tnsl.start + k * 128:
                              tnsl.start + (k + 1) * 128, :],
                    in_=tosbs[k][:])

        def emit_evacs(pi, pro_ps):
            # Evacuate the previous super's readout PSUM unscaled on the
            # boundary-idle DVE (ScalarE keeps the exp stream) plus the
            # denominator row, so the banks free without waiting on the
            # 1/s chain.
            osbs = []
            for k in range(4):
                osb = out_pool.tile([128, CV], f32, tag="osb",
                                    name=f"osb{pi}_{k}")
                nc.vector.tensor_copy(osb[:], pro_ps[k][:])
                osbs.append(osb)
            s_sb = row_pool.tile([1, NT], f32, tag="ssb", name=f"ssb{pi}")
            nc.vector.tensor_copy(s_sb[:], s_ps[:])
            return s_sb, osbs

        pending_tail = None
        prev = None
        for i in range(NSUP):
            nsl = slice(i * NT, (i + 1) * NT)
            ro_ps = None
            sacc = sacc_pool.tile([128, NT], f32r, tag="sacc",
                                  name=f"sacc{i}")
            for t in range(PAIRS):
                e = e_pool.tile([128, 2, NT], f8, tag="E", name=f"e{i}_{t}")
                for h, m in ((0, 2 * t), (1, 2 * t + 1)):
                    qk_ps = qk_ps_pool.tile([128, NT], f32, tag="qk_ps",
                                            name=f"qkps{i}_{t}_{h}")
                    nc.tensor.matmul(
                        qk_ps[:],
                        lhsT=mk_sb[:, m * 128:(m + 1) * 128],
                        rhs=qk_sb[:, nsl],
                        start=True, stop=True)
                    with nc.allow_low_precision(reason="fp8 softmax "
                                                "weights, tol 2e-2"):
                        nc.scalar.activation(
                            e[:, h, :], qk_ps[:],
                            mybir.ActivationFunctionType.Exp, scale=0.25)
                if t == 0:
                    # Boundary: previous super's evacuations go out after
                    # this super's first exps so ScalarE's exp stream is
                    # not delayed; only then allocate this super's
                    # readout banks (pool read-before-realloc order).
                    if prev is not None:
                        ps_sb, posbs = emit_evacs(prev[0], prev[1])
                        pending_tail = (prev[0], ps_sb, posbs, prev[2])
                    ro_ps = [ro_ps_pool.tile([128, CV], f32, tag=f"ro{k}",
                                             name=f"ro{k}_{i}")
                             for k in range(4)]
                if t < SPLIT:
                    with nc.allow_low_precision(reason="fp8 softmax "
                                                "sum, tol 2e-2"):
                        for h in (0, 1):
                            if t == 0 and h == 0:
                                nc.vector.tensor_copy(sacc[:], e[:, 0, :])
                            else:
                                nc.vector.tensor_add(sacc[:], e[:, h, :],
                                                     sacc[:])
                else:
                    nc.tensor.matmul(s_ps[0:1, :], lhsT=ones2[:, :, 0:1],
                                     rhs=e[:], perf_mode=DR,
                                     start=(t == SPLIT),
                                     stop=(t == PAIRS - 1))
                if t == SPLIT + 2:
                    # fold the DVE partial into the open accumulation
                    # group (in-order on PE, after the group's start)
                    nc.tensor.matmul(s_ps[0:1, :], lhsT=ones_col[:],
                                     rhs=sacc[:], start=False, stop=False,
                                     skip_group_check=True)
                if t == 3 and pending_tail is not None:
                    emit_tail(*pending_tail)
                    pending_tail = None
                for k in range(4):
                    nc.tensor.matmul(
                        ro_ps[k][:],
                        lhsT=e[:, :, k * 128:(k + 1) * 128],
                        rhs=mvt_sb[:, 2 * t:2 * t + 2, :],
                        perf_mode=DR,
                        start=(t == 0), stop=(t == PAIRS - 1))
            prev = (i, ro_ps, nsl)

        # Final super: fused tail. The 1/s chain starts as soon as the
        # denominator accumulation stops (while the last readout matmuls
        # still stream), and the evacuation applies the scale in one
        # pass, split across DVE and ScalarE.
        fi, fro, fnsl = prev
        s_sb = row_pool.tile([1, NT], f32, tag="ssb", name="ssb_f")
        nc.vector.tensor_copy(s_sb[:], s_ps[:])
        st = qk_ps_pool.tile([128, 4], f32, tag="qk_ps", name="st_f")
        for k in range(4):
            nc.tensor.matmul(st[:, k:k + 1],
                             lhsT=s_sb[0:1, k * 128:(k + 1) * 128],
                             rhs=one1[:], start=True, stop=True)
        inv_s = inv_pool.tile([128, 4], f32, tag="inv", name="inv_f")
        nc.vector.reciprocal(inv_s[:], st[:])
        for k in range(4):
            osb = out_pool.tile([128, CV], f32, tag="osb", name=f"osbf_{k}")
            with nc.allow_low_precision(reason="fp32 scale of fp32 data"):
                if k < 2:
                    nc.vector.tensor_scalar_mul(osb[:], fro[k][:],
                                                inv_s[:, k:k + 1])
                else:
                    nc.scalar.activation(
                        osb[:], fro[k][:],
                        mybir.ActivationFunctionType.Copy,
                        scale=inv_s[:, k:k + 1])
            nc.sync.dma_start(
                out=mem_d[fnsl.start + k * 128:fnsl.start + (k + 1) * 128, :],
                in_=osb[:])

    nc.compile()
    return nc


def _get_program():
    if "nc" not in _CACHE:
        _CACHE["nc"] = _build_program()
    return _CACHE["nc"]


def _make_in_maps(mk, qk, mv):
    import ml_dtypes

    mk = np.asarray(mk, dtype=np.float32)
    qk = np.asarray(qk, dtype=np.float32)
    mv = np.asarray(mv, dtype=np.float32)
    in_maps = []
    zpad = np.zeros((127 - CK, M), dtype=np.float32)
    ones_row = np.ones((1, N), dtype=np.float32)
    for b in range(B):
        mkf = mk[b].reshape(CK, M)
        # row 64 = 4 * (2 - |col|^2/8): exp-bias delivered via the matmul
        # (exp applies scale 0.25 afterwards); +2 centers fp8 E and
        # cancels against the denominator
        bias_row = (8.0 - 0.5 * (mkf * mkf).sum(axis=0))[None, :]
        mk_b = np.ascontiguousarray(
            np.concatenate([mkf, bias_row, zpad], axis=0))
        qk_b = np.ascontiguousarray(
            np.concatenate([qk[b].reshape(CK, N), ones_row, zpad], axis=0))
        # mvt[p, j*CV + c] = mv[b][c, j*128 + p], quantized to fp8e4
        mvt_b = np.ascontiguousarray(
            mv[b].reshape(CV, MCH, 128).transpose(2, 1, 0)
            .reshape(128, MCH * CV).astype(ml_dtypes.float8_e4m3))
        in_maps.append({"mk": mk_b, "qk": qk_b, "mvt": mvt_b})
    return in_maps


def kernel(mk, qk, mv, qv):
    qv = np.asarray(qv, dtype=np.float32)
    nc = _get_program()
    from concourse.bass_utils import run_bass_kernel_spmd

    in_maps = _make_in_maps(mk, qk, mv)
    res = run_bass_kernel_spmd(nc, in_maps, list(range(N_CORES)))
    mem = np.stack([res.results[b]["mem"].T for b in range(B)], axis=0)
    mem = np.ascontiguousarray(mem).reshape(B, CV, H, W)
    return np.concatenate([mem, qv], axis=1)


# revision 22
# speedup vs baseline: 1.0264x; 1.0264x over previous
"""MemoryReader kernel for Trainium2, data-parallel over batch across 8 cores.

Per batch element b (one NeuronCore each):
    mkf = mk[b] as [CK=64, M=4096], qkf = qk[b] as [CK, N=4096]
    aff[m, n] = (2 * mkf.T @ qkf - |mkf[:,m]|^2) / sqrt(CK)
    P = softmax over m
    mem[c, n]  = sum_m mv[b][c, m] * P[m, n]
    out[b] = concat([mem, qv[b]], channel axis)

Device kernel layout (per core), v3 (transposed fp8 DoubleRow readout):
    - QK^T matmuls produce logit chunks in [m-partition, n-free] layout,
      one [128, 512] PSUM bank per m-chunk, 3 rotating banks. The exp
      bias (2 - |mk col|^2/8, the +2 centers fp8 range and cancels in
      the softmax) rides in contraction row 64: mk row 64 holds
      8 - |col|^2/2 and qk row 64 holds 1.0, so the bias comes out of
      the matmul for free and ScalarE's exp needs no bias operand.
    - ScalarE computes E = exp(0.25 * psum) per chunk straight out of
      PSUM, writing float8e4 halves of a [128, 2, 512] pair tile.
    - Readout is TRANSPOSED: for each n-chunk k, out[n, c] accumulates
      lhsT = E-pair[:, :, 128k:128k+128] (stationary) against
      rhs = mv-pair [128, 2, 512] (moving) in fp8 DoubleRow mode
      (256 contraction rows per instruction). Output [n-part, c-free]
      makes the softmax 1/s a per-partition scalar.
    - Denominator: a DoubleRow ones-matmul accumulates s[n] per super in
      one [1, 512] PSUM row; rows 0/32 of one bank alternate between
      supers so the next super never waits on the tail chain.
    - PSUM budget: 3 (qk) + 4 (readout) + 1 (denominator) = 8 banks.
    - Tail per super (emitted a few pairs into the next super): ScalarE
      copies s to SBUF, four K=1 matmuls transpose it to [128, 4], DVE
      reciprocal + per-partition scale of the evacuated readout, DMA out.
    - mv^T is quantized to fp8e4 and laid out host-side. qv never
      touches the device. Output is [N, CV]; host transposes back.
"""

import os
import sys

import numpy as np

B, CK, CV, H, W = 8, 64, 512, 64, 64
M = H * W          # memory positions per batch element
N = H * W          # query positions
NT = 512           # n-super-tile width (columns per softmax pass)
NSUP = N // NT     # 8 n-super-tiles
MCH = M // 128     # 32 m-chunks
PAIRS = MCH // 2   # 16 m-chunk pairs (DoubleRow contracts 256 rows)
N_CORES = 8

_CACHE = {}


def _build_program():
    sys.path.insert(0, "/opt/trn_rl_repo")
    from contextlib import ExitStack

    import concourse.tile as tile
    from concourse import bacc, mybir

    dt = mybir.dt
    f32 = dt.float32
    f32r = dt.float32r
    f8 = dt.float8e4
    DR = mybir.MatmulPerfMode.DoubleRow

    nc = bacc.Bacc("TRN2", target_bir_lowering=False, debug=False,
                   num_devices=N_CORES)

    mk_d = nc.dram_tensor("mk", [128, M], f32r, kind="ExternalInput").ap()
    qk_d = nc.dram_tensor("qk", [128, N], f32r, kind="ExternalInput").ap()
    mvt_d = nc.dram_tensor("mvt", [128, MCH * CV], f8,
                           kind="ExternalInput").ap()
    mem_d = nc.dram_tensor("mem", [N, CV], f32, kind="ExternalOutput").ap()

    with tile.TileContext(nc) as tc, ExitStack() as ctx:
        sing = ctx.enter_context(tc.tile_pool(name="sing", bufs=1))
        e_pool = ctx.enter_context(tc.tile_pool(name="E", bufs=12))
        sacc_pool = ctx.enter_context(tc.tile_pool(name="sacc", bufs=2))
        row_pool = ctx.enter_context(tc.tile_pool(name="row", bufs=2))
        inv_pool = ctx.enter_context(tc.tile_pool(name="inv", bufs=2))
        out_pool = ctx.enter_context(tc.tile_pool(name="out", bufs=8))
        qk_ps_pool = ctx.enter_context(
            tc.tile_pool(name="qkps", bufs=3, space="PSUM"))
        ro_ps_pool = ctx.enter_context(
            tc.tile_pool(name="rops", bufs=1, space="PSUM"))
        s_ps_pool = ctx.enter_context(
            tc.tile_pool(name="sps", bufs=1, space="PSUM"))

        # PE warmup: the PE activity monitor starts throttled at 1.2 GHz
        # and needs ~3.4us of sustained matmul activity to unthrottle.
        # Burn dummy matmuls while the input DMAs stream so the real
        # matmuls start at 2.4 GHz.
        warm_sb = sing.tile([128, 128], f32)
        nc.vector.memset(warm_sb[:], 1.0)
        warm_ps = qk_ps_pool.tile([128, NT], f32, tag="qk_ps", name="warm_ps")
        for w in range(16):
            nc.tensor.matmul(warm_ps[:, 0:128], lhsT=warm_sb[:],
                             rhs=warm_sb[:], start=True, stop=True)

        # Resident inputs. mk/qk carry CK=64 data rows, the exp-bias /
        # ones row at 64, and zero padding to K=128 (K=64 matmuls run at
        # the throttled PE clock, so padded K=128 is 2x faster). DMAs
        # ordered so the tensors gating the first matmuls arrive first.
        mk_sb = sing.tile([128, M], f32r)
        qk_sb = sing.tile([128, N], f32r)
        mvt_sb = sing.tile([128, MCH, CV], f8)
        nc.sync.dma_start(out=qk_sb[:, 0:NT], in_=qk_d[:, 0:NT])
        nc.sync.dma_start(out=mk_sb[:, 0:1024], in_=mk_d[:, 0:1024])
        nc.sync.dma_start(
            out=mvt_sb[:, 0:8, :],
            in_=mvt_d[:, 0:8 * CV].rearrange("p (j c) -> p j c", c=CV))
        for g in range(1, 4):
            gs = slice(g * 1024, (g + 1) * 1024)
            nc.sync.dma_start(out=mk_sb[:, gs], in_=mk_d[:, gs])
        for g in range(1, 4):
            gs = slice(g * 8 * CV, (g + 1) * 8 * CV)
            nc.sync.dma_start(
                out=mvt_sb[:, g * 8:(g + 1) * 8, :],
                in_=mvt_d[:, gs].rearrange("p (j c) -> p j c", c=CV))
        nc.sync.dma_start(out=qk_sb[:, NT:N], in_=qk_d[:, NT:N])

        # Ones operands: fp8 pair-column for the DoubleRow denominator
        # matmul, fp32r single element for the s-transpose matmuls.
        ones_f32 = sing.tile([128, 2, 16], f32)
        nc.vector.memset(ones_f32[:], 1.0)
        ones2 = sing.tile([128, 2, 16], f8)
        with nc.allow_low_precision(reason="exact value 1.0 in fp8"):
            nc.vector.tensor_copy(ones2[:], ones_f32[:])
        one1 = sing.tile([1, 1], f32)
        nc.vector.memset(one1[:], 1.0)
        ones_col_f32 = sing.tile([128, 1], f32)
        nc.vector.memset(ones_col_f32[:], 1.0)
        ones_col = sing.tile([128, 1], f32r)
        nc.vector.tensor_copy(ones_col[:], ones_col_f32[:].bitcast(f32r))

        s_ps = s_ps_pool.tile([1, NT], f32, tag="s_ps", name="s_ps")

        # Softmax-sum split: pairs 0..SPLIT-1 accumulate on VectorE from
        # the fp8 E tiles (sacc), pairs SPLIT..15 on PE via DoubleRow
        # ones-matmuls; a K=128 ones-matmul folds sacc's partition axis
        # into row 32 of the denominator bank.
        SPLIT = 12

        def emit_tail(ti, ts_sb, tosbs, tnsl):
            # 1/s chain and final scaling for super `ti`, emitted a few
            # pairs into the NEXT super so the boundary engines stay
            # clear while the chain resolves.
            st = qk_ps_pool.tile([128, 4], f32, tag="qk_ps",
                                 name=f"st{ti}")
            for k in range(4):
                nc.tensor.matmul(
                    st[:, k:k + 1],
                    lhsT=ts_sb[0:1, k * 128:(k + 1) * 128],
                    rhs=one1[:], start=True, stop=True)
            inv_s = inv_pool.tile([128, 4], f32, tag="inv",
                                  name=f"inv{ti}")
            nc.vector.reciprocal(inv_s[:], st[:])
            with nc.allow_low_precision(reason="fp32 scale of fp32 data"):
                for k in range(4):
                    nc.vector.tensor_scalar_mul(
                        tosbs[k][:], tosbs[k][:], inv_s[:, k:k + 1])
            for k in range(4):
                nc.sync.dma_start(
                    out=mem_d[tnsl.start + k * 128:
                              tnsl.start + (k + 1) * 128, :],
                    in_=tosbs[k][:])

        def emit_evacs(pi, pro_ps):
            # Evacuate the previous super's readout PSUM unscaled on the
            # boundary-idle DVE (ScalarE keeps the exp stream) plus the
            # denominator row, so the banks free without waiting on the
            # 1/s chain.
            osbs = []
            for k in range(4):
                osb = out_pool.tile([128, CV], f32, tag="osb",
                                    name=f"osb{pi}_{k}")
                nc.vector.tensor_copy(osb[:], pro_ps[k][:])
                osbs.append(osb)
            s_sb = row_pool.tile([1, NT], f32, tag="ssb", name=f"ssb{pi}")
            nc.vector.tensor_copy(s_sb[:], s_ps[:])
            return s_sb, osbs

        pending_tail = None
        prev = None
        for i in range(NSUP):
            nsl = slice(i * NT, (i + 1) * NT)
            ro_ps = None
            sacc = sacc_pool.tile([128, NT], f32r, tag="sacc",
                                  name=f"sacc{i}")
            # Readout matmuls for pair t-1 interleave between pair t's
            # QK matmuls: DoubleRow LDWEIGHTS (256 cols, ~213ns) only
            # just hides under a DR matmul, so back-to-back readout
            # chains are LDWEIGHTS-bound; slotting the cheap-LDW QK
            # matmuls between them recovers the slack.
            def emit_ro(pt, pe):
                return [nc.tensor.matmul(
                    ro_ps[k][:],
                    lhsT=pe[:, :, k * 128:(k + 1) * 128],
                    rhs=mvt_sb[:, 2 * pt:2 * pt + 2, :],
                    perf_mode=DR,
                    start=(pt == 0), stop=(pt == PAIRS - 1))
                    for k in ks]
            prev_e = None
            for t in range(PAIRS + 1):
                e = None
                if t < PAIRS:
                    e = e_pool.tile([128, 2, NT], f8, tag="E",
                                    name=f"e{i}_{t}")
                    m = 2 * t
                    qk_ps = qk_ps_pool.tile([128, NT], f32, tag="qk_ps",
                                            name=f"qkps{i}_{t}_0")
                    nc.tensor.matmul(
                        qk_ps[:],
                        lhsT=mk_sb[:, m * 128:(m + 1) * 128],
                        rhs=qk_sb[:, nsl],
                        start=True, stop=True)
                    with nc.allow_low_precision(reason="fp8 softmax "
                                                "weights, tol 2e-2"):
                        nc.scalar.activation(
                            e[:, 0, :], qk_ps[:],
                            mybir.ActivationFunctionType.Exp, scale=0.25)
                if t == 1:
                    # Boundary: previous super's evacuations go out after
                    # this super's first exps so ScalarE's exp stream is
                    # not delayed; only then allocate this super's
                    # readout banks (pool read-before-realloc order).
                    if prev is not None:
                        ps_sb, posbs = emit_evacs(prev[0], prev[1])
                        pending_tail = (prev[0], ps_sb, posbs, prev[2])
                    ro_ps = [ro_ps_pool.tile([128, CV], f32, tag=f"ro{k}",
                                             name=f"ro{k}_{i}")
                             for k in range(4)]
                if prev_e is not None:
                    for ks in ((0, 1),):
                        emit_ro(t - 1, prev_e)
                if t < PAIRS:
                    m = 2 * t + 1
                    qk_ps = qk_ps_pool.tile([128, NT], f32, tag="qk_ps",
                                            name=f"qkps{i}_{t}_1")
                    nc.tensor.matmul(
                        qk_ps[:],
                        lhsT=mk_sb[:, m * 128:(m + 1) * 128],
                        rhs=qk_sb[:, nsl],
                        start=True, stop=True)
                    with nc.allow_low_precision(reason="fp8 softmax "
                                                "weights, tol 2e-2"):
                        nc.scalar.activation(
                            e[:, 1, :], qk_ps[:],
                            mybir.ActivationFunctionType.Exp, scale=0.25)
                if prev_e is not None:
                    for ks in ((2, 3),):
                        emit_ro(t - 1, prev_e)
                if t < PAIRS:
                    if t < SPLIT:
                        with nc.allow_low_precision(reason="fp8 softmax "
                                                    "sum, tol 2e-2"):
                            for h in (0, 1):
                                if t == 0 and h == 0:
                                    nc.vector.tensor_copy(sacc[:],
                                                          e[:, 0, :])
                                else:
                                    nc.vector.tensor_add(
                                        sacc[:], e[:, h, :], sacc[:])
                    else:
                        nc.tensor.matmul(s_ps[0:1, :],
                                         lhsT=ones2[:, :, 0:1],
                                         rhs=e[:], perf_mode=DR,
                                         start=(t == SPLIT),
                                         stop=(t == PAIRS - 1))
                if t == SPLIT + 2:
                    # fold the DVE partial into the open accumulation
                    # group (in-order on PE, after the group's start)
                    nc.tensor.matmul(s_ps[0:1, :], lhsT=ones_col[:],
                                     rhs=sacc[:], start=False, stop=False,
                                     skip_group_check=True)
                if t == 3 and pending_tail is not None:
                    emit_tail(*pending_tail)
                    pending_tail = None
                prev_e = e
            prev = (i, ro_ps, nsl)

        # Final super: fused tail. The 1/s chain starts as soon as the
        # denominator accumulation stops (while the last readout matmuls
        # still stream), and the evacuation applies the scale in one
        # pass, split across DVE and ScalarE.
        fi, fro, fnsl = prev
        s_sb = row_pool.tile([1, NT], f32, tag="ssb", name="ssb_f")
        nc.vector.tensor_copy(s_sb[:], s_ps[:])
        st = qk_ps_pool.tile([128, 4], f32, tag="qk_ps", name="st_f")
        for k in range(4):
            nc.tensor.matmul(st[:, k:k + 1],
                             lhsT=s_sb[0:1, k * 128:(k + 1) * 128],
                             rhs=one1[:], start=True, stop=True)
        inv_s = inv_pool.tile([128, 4], f32, tag="inv", name="inv_f")
        nc.vector.reciprocal(inv_s[:], st[:])
        for k in range(4):
            osb = out_pool.tile([128, CV], f32, tag="osb", name=f"osbf_{k}")
            with nc.allow_low_precision(reason="fp32 scale of fp32 data"):
                if k < 2:
                    nc.vector.tensor_scalar_mul(osb[:], fro[k][:],
                                                inv_s[:, k:k + 1])
                else:
                    nc.scalar.activation(
                        osb[:], fro[k][:],
                        mybir.ActivationFunctionType.Copy,
                        scale=inv_s[:, k:k + 1])
            nc.sync.dma_start(
                out=mem_d[fnsl.start + k * 128:fnsl.start + (k + 1) * 128, :],
                in_=osb[:])

    nc.compile()
    return nc


def _get_program():
    if "nc" not in _CACHE:
        _CACHE["nc"] = _build_program()
    return _CACHE["nc"]


def _make_in_maps(mk, qk, mv):
    import ml_dtypes

    mk = np.asarray(mk, dtype=np.float32)
    qk = np.asarray(qk, dtype=np.float32)
    mv = np.asarray(mv, dtype=np.float32)
    in_maps = []
    zpad = np.zeros((127 - CK, M), dtype=np.float32)
    ones_row = np.ones((1, N), dtype=np.float32)
    for b in range(B):
        mkf = mk[b].reshape(CK, M)
        # row 64 = 4 * (2 - |col|^2/8): exp-bias delivered via the matmul
        # (exp applies scale 0.25 afterwards); +2 centers fp8 E and
        # cancels against the denominator
        bias_row = (8.0 - 0.5 * (mkf * mkf).sum(axis=0))[None, :]
        mk_b = np.ascontiguousarray(
            np.concatenate([mkf, bias_row, zpad], axis=0))
        qk_b = np.ascontiguousarray(
            np.concatenate([qk[b].reshape(CK, N), ones_row, zpad], axis=0))
        # mvt[p, j*CV + c] = mv[b][c, j*128 + p], quantized to fp8e4
        mvt_b = np.ascontiguousarray(
            mv[b].reshape(CV, MCH, 128).transpose(2, 1, 0)
            .reshape(128, MCH * CV).astype(ml_dtypes.float8_e4m3))
        in_maps.append({"mk": mk_b, "qk": qk_b, "mvt": mvt_b})
    return in_maps


def kernel(mk, qk, mv, qv):
    qv = np.asarray(qv, dtype=np.float32)
    nc = _get_program()
    from concourse.bass_utils import run_bass_kernel_spmd

    in_maps = _make_in_maps(mk, qk, mv)
    res = run_bass_kernel_spmd(nc, in_maps, list(range(N_CORES)))
    mem = np.stack([res.results[b]["mem"].T for b in range(B)], axis=0)
    mem = np.ascontiguousarray(mem).reshape(B, CV, H, W)
    return np.concatenate([mem, qv], axis=1)
